# revision 3
# baseline (speedup 1.0000x reference)
"""Trainium2 Bass kernel for nn_CausalBankModel (V=32000, E=256, M=256, T=1024,
B=2, H=1024, W=8) on 8 NeuronCores.

The per-dispatch cost in this environment is dominated by shipping
ExternalInput/ExternalOutput bytes through the PJRT tunnel, so the design
minimizes per-dispatch I/O:

- All weights (in_proj, W1s, W2s, biases, decay-scan constants) are baked into
  the NEFF as inline Const tensors -> loaded to device HBM once at model load,
  zero per-dispatch cost. One shared SPMD program for all 8 cores.
- The embedding gather x = emb[chars] happens on host; the kernel ships only
  the transposed padded activations xt (bf16, ~1 MB), identical to all cores.
- Row sharding: core c computes rows [c*256, (c+1)*256) of the B*T=2048 rows
  through both MLP paths over the FULL vocab, so logit statistics (entropy,
  max, variance) are core-local: no collectives at all. The causal decay scan
  (cheap) is computed redundantly on every core from the full xt.
- Core selection is data-driven, not program-driven: each core receives a tiny
  one-hot mask [128, 8]; "my 256 rows" of states/x are extracted with 8 masked
  multiply-accumulate ops (exact for 0/1 masks), keeping one static program.
- Output is the core's [256, 32000] mixed logits in f16 (16.4 MB vs 32.8 f32).

Scan: within a 128-step chunk, states = diag(d^i) @ TriU @ diag(d^-j) via one
128x128 triangular matmul per chunk per 128-mode half, plus a per-partition
carry add (exact in f32; worst-case rescale 0.85^-127 ~ 8.8e8 is inside f32
range). Stats use ScalarE activation accum_out (fused free-axis sum); max uses
DVE reduce_max. Sum-of-logits comes from an extra W2 column holding row sums.
"""

import sys

import numpy as np

sys.path.insert(0, "/opt/trn_rl_repo")

import ml_dtypes  # noqa: E402

from concourse import bacc, mybir, tile  # noqa: E402
from concourse.bass_utils import run_bass_kernel_spmd  # noqa: E402

F32 = mybir.dt.float32
BF16 = mybir.dt.bfloat16
F16 = mybir.dt.float16
AF = mybir.ActivationFunctionType
ALU = mybir.AluOpType
X_AXIS = mybir.AxisListType.X

V, E, M, T, B, H, W = 32000, 256, 256, 1024, 2, 1024, 8
N_CORES = 8
CORE_IDS = list(range(N_CORES))
NR = B * T              # 2048 rows
RPC = NR // N_CORES     # 256 rows per core
HBLK = H // 128         # 8 hidden blocks
LPAD = T + W - 1        # 1031 padded columns per batch in x_T
CL = B * LPAD           # 2062 total xt columns
CHUNK = 128             # scan chunk length
NCH = T // CHUNK        # 8 chunks per batch
VP = 32256              # padded vocab (63 x 512); col 32000 = row-sum column
VBW = 1536              # vocab cols per streamed W2 block (3 psum chunks)
NVB = VP // VBW         # 21 blocks
VLAST = V - (NVB - 1) * VBW  # 1280 real cols in the last block
XW = W - 1 + RPC        # 263 xt cols a core needs (7 history + 256)


def _bf(a):
    return np.ascontiguousarray(np.asarray(a, np.float32).astype(ml_dtypes.bfloat16))


def _cbase(c):
    b, q = divmod(c, N_CORES // B)
    return b * LPAD + q * RPC


def build_program(decays_np, gate_w, gate_b, weights):
    """Build the per-core Bass program. ALL weights are baked into the NEFF
    as inline constants; only xt + the row mask arrive per dispatch."""
    nc = bacc.Bacc(None, target_bir_lowering=False)

    xt_d = nc.dram_tensor("xt", [2, 128, CL], BF16, kind="ExternalInput")
    msk_d = nc.dram_tensor("msk", [128, N_CORES], F32, kind="ExternalInput")
    out_d = nc.dram_tensor("out", [2, 128, V], F16, kind="ExternalOutput")

    use_b2 = weights["use_b2"]

    # host-precomputed scan constants
    j = np.arange(CHUNK)
    d64 = np.asarray(decays_np, dtype=np.float64)
    tri = np.triu(np.ones((CHUNK, CHUNK), np.float32))           # [j, i] j<=i
    dneg = (d64[None, :] ** (-j[:, None])).astype(np.float32)    # [j=128, M]
    dpow = (d64[:, None] ** j[None, :]).astype(np.float32)       # [M, i=128]
    tri_t = nc.inline_tensor(tri, "tri")
    dneg_t = nc.inline_tensor(dneg, "dneg")
    dpow_t = nc.inline_tensor(dpow, "dpow")
    dvec_t = nc.inline_tensor(
        np.ascontiguousarray(d64.astype(np.float32).reshape(2, 128, 1)), "dvec"
    )
    inp_t = nc.inline_tensor(weights["inp"], "inp")      # [2,128,M] bf16
    w1l_t = nc.inline_tensor(weights["w1l"], "w1l")      # [4,128,H] bf16
    w1o_t = nc.inline_tensor(weights["w1o"], "w1o")      # [16,128,H] bf16
    b1l_t = nc.inline_tensor(weights["b1l"], "b1l")      # [HBLK,128,1] f32
    b1o_t = nc.inline_tensor(weights["b1o"], "b1o")
    w2l_t = nc.inline_tensor(weights["w2l"], "w2l")      # [HBLK,128,VP] bf16
    w2o_t = nc.inline_tensor(weights["w2o"], "w2o")
    b2_t = nc.inline_tensor(weights["b2"], "b2") if use_b2 else None

    with tile.TileContext(nc) as tc, tile.ExitStack() as top:
        sb = top.enter_context(tc.tile_pool(name="sb", bufs=1))
        dr = top.enter_context(tc.tile_pool(name="dr", bufs=1, space="DRAM"))

        # ---------- resident sbuf state ----------
        xtbf = [sb.tile([128, CL], BF16, tag=f"xtbf{e}", name=f"xtbf{e}")
                for e in range(2)]
        st_bf = [sb.tile([128, NR], BF16, tag=f"stbf{m}", name=f"stbf{m}")
                 for m in range(2)]
        msk_s = sb.tile([128, N_CORES], F32, tag="msk")
        xt_my = [sb.tile([128, XW], BF16, tag=f"xtmy{e}", name=f"xtmy{e}")
                 for e in range(2)]
        st_my = [sb.tile([128, RPC], BF16, tag=f"stmy{m}", name=f"stmy{m}")
                 for m in range(2)]
        ht = [sb.tile([128, HBLK, RPC], BF16, tag=f"ht{p}", name=f"ht{p}")
              for p in range(2)]
        ones_s = sb.tile([1, 128], BF16, tag="ones1")
        if use_b2:
            nc.vector.memset(ones_s[:], 1.0)
        stats = {}
        for p in range(2):
            for nm in ("se", "sa", "sq", "mx"):
                stats[(p, nm)] = sb.tile([128, 2, NVB], F32, tag=f"st{p}{nm}",
                                         name=f"st{p}{nm}")
            stats[(p, "sl")] = sb.tile([128, 2], F32, tag=f"st{p}sl",
                                       name=f"st{p}sl")
        ldram = [dr.tile([2, 128, VP], BF16, name=f"ldram{p}") for p in range(2)]

        for e in range(2):
            nc.sync.dma_start(out=xtbf[e][:], in_=xt_d[e])
        nc.sync.dma_start(out=msk_s[:], in_=msk_d[:])

        # ---------- causal decay scan over all rows (redundant per core) ----
        scn_cm = tile.ExitStack()
        scn = scn_cm.enter_context(tc.tile_pool(name="scn", bufs=1))
        tri_s = scn.tile([128, 128], F32, tag="tri")
        dneg_s = scn.tile([128, M], F32, tag="dneg")
        dpow_s = [scn.tile([128, 128], F32, tag=f"dpow{m}", name=f"dpow{m}")
                  for m in range(2)]
        dvec_s = [scn.tile([128, 1], F32, tag=f"dvec{m}", name=f"dvec{m}")
                  for m in range(2)]
        inp_s = scn.tile([128, 2, M], BF16, tag="inp")
        nc.sync.dma_start(out=tri_s[:], in_=tri_t[:])
        nc.sync.dma_start(out=dneg_s[:], in_=dneg_t[:])
        for m in range(2):
            nc.sync.dma_start(out=dpow_s[m][:], in_=dpow_t[m * 128:(m + 1) * 128, :])
            nc.sync.dma_start(out=dvec_s[m][:], in_=dvec_t[m])
            nc.sync.dma_start(out=inp_s[:, m, :], in_=inp_t[m])

        with (
            tc.tile_pool(name="cv", bufs=4) as cv,
            tc.tile_pool(name="pd", bufs=2, space="PSUM") as pd,
            tc.tile_pool(name="pc", bufs=4, space="PSUM") as pc,
        ):
            carry = {}
            for b in range(B):
                for m in range(2):
                    cz = cv.tile([128, 1], F32, tag=f"car{b}{m}")
                    nc.vector.memset(cz[:], 0.0)
                    carry[(b, m)] = cz
            for c in range(NCH):
                for b in range(B):
                    col = b * LPAD + (W - 1) + c * CHUNK
                    psd = pd.tile([128, M], F32, tag="psd")
                    for e in range(2):
                        nc.tensor.matmul(
                            psd[:], xtbf[e][:, col:col + 128], inp_s[:, e, :],
                            start=(e == 0), stop=(e == 1),
                        )
                    scaled = cv.tile([128, M], F32, tag="scaled")
                    nc.vector.tensor_mul(scaled[:], psd[:], dneg_s[:])
                    n0 = b * T + c * CHUNK
                    for m in range(2):
                        psc = pc.tile([128, 128], F32, tag="psc")
                        nc.tensor.matmul(
                            psc[:], scaled[:, m * 128:(m + 1) * 128], tri_s[:],
                            start=True, stop=True,
                        )
                        nc.vector.tensor_scalar_add(psc[:], psc[:], carry[(b, m)][:])
                        nc.vector.tensor_mul(
                            st_bf[m][:, n0:n0 + CHUNK], psc[:], dpow_s[m][:]
                        )
                        cn = cv.tile([128, 1], F32, tag=f"car{b}{m}")
                        nc.vector.tensor_mul(
                            cn[:], psc[:, 127:128], dpow_s[m][:, 127:128]
                        )
                        nc.vector.tensor_mul(cn[:], cn[:], dvec_s[m][:])
                        carry[(b, m)] = cn
        scn_cm.close()

        # ---------- masked row-select: my 256 rows (data-driven, no branches)
        for e in range(2):
            nc.vector.tensor_scalar_mul(
                xt_my[e][:], xtbf[e][:, _cbase(0):_cbase(0) + XW], msk_s[:, 0:1]
            )
            for c in range(1, N_CORES):
                nc.vector.scalar_tensor_tensor(
                    out=xt_my[e][:], in0=xtbf[e][:, _cbase(c):_cbase(c) + XW],
                    scalar=msk_s[:, c:c + 1], in1=xt_my[e][:],
                    op0=ALU.mult, op1=ALU.add,
                )
        for m in range(2):
            nc.vector.tensor_scalar_mul(
                st_my[m][:], st_bf[m][:, 0:RPC], msk_s[:, 0:1]
            )
            for c in range(1, N_CORES):
                nc.vector.scalar_tensor_tensor(
                    out=st_my[m][:], in0=st_bf[m][:, c * RPC:(c + 1) * RPC],
                    scalar=msk_s[:, c:c + 1], in1=st_my[m][:],
                    op0=ALU.mult, op1=ALU.add,
                )

        # ---------- W1 (hidden layer) for my rows, both paths ----------
        for path in range(2):
            w1_t, b1_t = (w1l_t, b1l_t) if path == 0 else (w1o_t, b1o_t)
            nk1 = 4 if path == 0 else 16

            def rhs_for(kk):
                if path == 0:
                    if kk < 2:
                        return st_my[kk][:]
                    return xt_my[kk - 2][:, W - 1:W - 1 + RPC]
                o, e = divmod(kk, 2)
                return xt_my[e][:, W - 1 - o:W - 1 - o + RPC]

            with (
                tc.tile_pool(name=f"w1p{path}", bufs=1) as w1p,
                tc.tile_pool(name=f"psh{path}", bufs=2, space="PSUM") as psh,
            ):
                w1_s = w1p.tile([128, nk1, H], BF16, tag="w1")
                for kk in range(nk1):
                    nc.sync.dma_start(out=w1_s[:, kk, :], in_=w1_t[kk])
                b1_s = w1p.tile([128, HBLK], F32, tag="b1")
                for hh in range(HBLK):
                    nc.sync.dma_start(out=b1_s[:, hh:hh + 1], in_=b1_t[hh])
                for hh in range(HBLK):
                    ps = psh.tile([128, RPC], F32, tag="ph", bufs=2)
                    for kk in range(nk1):
                        nc.tensor.matmul(
                            ps[:], w1_s[:, kk, hh * 128:(hh + 1) * 128],
                            rhs_for(kk), start=(kk == 0), stop=(kk == nk1 - 1),
                        )
                    nc.scalar.activation(
                        ht[path][:, hh, :], ps[:], AF.Relu,
                        bias=b1_s[:, hh:hh + 1],
                    )

        # ---------- W2 stream + logit stats, both paths ----------
        for path in range(2):
            w2_t = w2l_t if path == 0 else w2o_t
            s_se, s_sa = stats[(path, "se")], stats[(path, "sa")]
            s_sq, s_mx = stats[(path, "sq")], stats[(path, "mx")]
            s_sl = stats[(path, "sl")]
            ld = ldram[path]
            with (
                tc.tile_pool(name=f"w2p{path}", bufs=2) as w2p,
                tc.tile_pool(name=f"stp{path}", bufs=2) as stp,
                tc.tile_pool(name=f"pw{path}", bufs=2, space="PSUM") as pw,
            ):
                if use_b2:
                    b2_s = w2p.tile([1, VP], BF16, tag="b2", bufs=1)
                    nc.sync.dma_start(out=b2_s[:], in_=b2_t[path])
                for vb in range(NVB):
                    w2s = w2p.tile([128, HBLK, VBW], BF16, tag="w2s", bufs=2)
                    for hh in range(HBLK):
                        nc.sync.dma_start(
                            out=w2s[:, hh, :],
                            in_=w2_t[hh, :, vb * VBW:(vb + 1) * VBW],
                        )
                    w = VBW if vb < NVB - 1 else VLAST
                    for rb in range(2):
                        stage = stp.tile([128, VBW], BF16, tag="stage", bufs=3)
                        for ch in range(3):
                            psl = pw.tile([128, 512], F32, tag=f"pl{ch}",
                                          name=f"pl{ch}", bufs=2)
                            for hh in range(HBLK):
                                nc.tensor.matmul(
                                    psl[:],
                                    ht[path][:, hh, rb * 128:(rb + 1) * 128],
                                    w2s[:, hh, ch * 512:(ch + 1) * 512],
                                    start=(hh == 0),
                                    stop=(hh == HBLK - 1) and not use_b2,
                                )
                            if use_b2:
                                nc.tensor.matmul(
                                    psl[:], ones_s[:],
                                    b2_s[:, vb * VBW + ch * 512:
                                         vb * VBW + (ch + 1) * 512],
                                    start=False, stop=True,
                                )
                            nc.vector.tensor_copy(
                                stage[:, ch * 512:(ch + 1) * 512], psl[:]
                            )
                        et = stp.tile([128, VBW], BF16, tag="et", bufs=2)
                        nc.scalar.activation(
                            et[:, :w], stage[:, :w], AF.Exp,
                            accum_out=s_se[:, rb, vb:vb + 1],
                        )
                        dump = stp.tile([128, VBW], BF16, tag="dump", bufs=1)
                        nc.scalar.activation(
                            dump[:, :w], stage[:, :w], AF.Square,
                            accum_out=s_sq[:, rb, vb:vb + 1],
                        )
                        le = stp.tile([128, VBW], BF16, tag="le", bufs=2)
                        nc.vector.tensor_mul(le[:, :w], stage[:, :w], et[:, :w])
                        nc.scalar.activation(
                            dump[:, :w], le[:, :w], AF.Identity,
                            accum_out=s_sa[:, rb, vb:vb + 1],
                        )
                        nc.vector.tensor_reduce(
                            s_mx[:, rb, vb:vb + 1], stage[:, :w],
                            axis=X_AXIS, op=ALU.max,
                        )
                        if vb == NVB - 1:
                            nc.vector.tensor_copy(
                                s_sl[:, rb:rb + 1], stage[:, VLAST:VLAST + 1]
                            )
                        nc.sync.dma_start(
                            out=ld[rb, :, vb * VBW:(vb + 1) * VBW], in_=stage[:]
                        )

        # ---------- gate from local stats ----------
        gate = sb.tile([128, 2], F32, tag="gate")
        with tc.tile_pool(name="gtp", bufs=1) as gtp:
            feats = []
            for p in range(2):
                Sg = gtp.tile([128, 2], F32, tag=f"Sg{p}")
                Ag = gtp.tile([128, 2], F32, tag=f"Ag{p}")
                Qg = gtp.tile([128, 2], F32, tag=f"Qg{p}")
                Mg = gtp.tile([128, 2], F32, tag=f"Mg{p}")
                for rb in range(2):
                    nc.vector.tensor_reduce(
                        Sg[:, rb:rb + 1], stats[(p, "se")][:, rb, :],
                        axis=X_AXIS, op=ALU.add)
                    nc.vector.tensor_reduce(
                        Ag[:, rb:rb + 1], stats[(p, "sa")][:, rb, :],
                        axis=X_AXIS, op=ALU.add)
                    nc.vector.tensor_reduce(
                        Qg[:, rb:rb + 1], stats[(p, "sq")][:, rb, :],
                        axis=X_AXIS, op=ALU.add)
                    nc.vector.tensor_reduce(
                        Mg[:, rb:rb + 1], stats[(p, "mx")][:, rb, :],
                        axis=X_AXIS, op=ALU.max)
                Lg = stats[(p, "sl")]
                rS = gtp.tile([128, 2], F32, tag=f"rS{p}")
                nc.vector.reciprocal(rS[:], Sg[:])
                AoS = gtp.tile([128, 2], F32, tag=f"AoS{p}")
                nc.vector.tensor_mul(AoS[:], Ag[:], rS[:])
                lnS = gtp.tile([128, 2], F32, tag=f"lnS{p}")
                nc.scalar.activation(lnS[:], Sg[:], AF.Ln)
                ent = gtp.tile([128, 2], F32, tag=f"ent{p}")
                nc.vector.tensor_sub(ent[:], lnS[:], AoS[:])
                mean = gtp.tile([128, 2], F32, tag=f"mean{p}")
                nc.vector.tensor_scalar_mul(mean[:], Lg[:], 1.0 / V)
                m2 = gtp.tile([128, 2], F32, tag=f"m2{p}")
                nc.vector.tensor_mul(m2[:], mean[:], mean[:])
                var = gtp.tile([128, 2], F32, tag=f"var{p}")
                nc.vector.tensor_scalar_mul(var[:], Qg[:], 1.0 / V)
                nc.vector.tensor_sub(var[:], var[:], m2[:])
                feats += [ent, Mg, var]

            acc = gtp.tile([128, 2], F32, tag="gacc")
            nc.vector.tensor_scalar_mul(acc[:], feats[0][:], float(gate_w[0]))
            for i in range(1, 6):
                nc.vector.scalar_tensor_tensor(
                    out=acc[:], in0=feats[i][:], scalar=float(gate_w[i]),
                    in1=acc[:], op0=ALU.mult, op1=ALU.add,
                )
            nc.scalar.activation(
                gate[:], acc[:], AF.Sigmoid, bias=float(gate_b), scale=1.0
            )

        # ---------- gated mix -> f16 output ----------
        with tc.tile_pool(name="mx", bufs=3) as mxp:
            for rb in range(2):
                for vb in range(NVB):
                    w = VBW if vb < NVB - 1 else VLAST
                    lin_s = mxp.tile([128, VBW], BF16, tag="lin")
                    loc_s = mxp.tile([128, VBW], BF16, tag="loc")
                    nc.sync.dma_start(
                        out=lin_s[:, :w], in_=ldram[0][rb, :, vb * VBW:vb * VBW + w])
                    nc.sync.dma_start(
                        out=loc_s[:, :w], in_=ldram[1][rb, :, vb * VBW:vb * VBW + w])
                    d = mxp.tile([128, VBW], BF16, tag="d")
                    nc.vector.tensor_sub(d[:, :w], lin_s[:, :w], loc_s[:, :w])
                    o = mxp.tile([128, VBW], F16, tag="o")
                    nc.vector.scalar_tensor_tensor(
                        out=o[:, :w], in0=d[:, :w], scalar=gate[:, rb:rb + 1],
                        in1=loc_s[:, :w], op0=ALU.mult, op1=ALU.add,
                    )
                    nc.sync.dma_start(
                        out=out_d[rb, :, vb * VBW:vb * VBW + w], in_=o[:, :w]
                    )

    nc.compile()
    return nc


def _pack_weights(in_proj, lin_W1, lin_b1, lin_W2, lin_b2,
                  loc_W1, loc_b1, loc_W2, loc_b2):
    def w2pad(w2, b2):
        w = np.zeros((H, VP), np.float32)
        w[:, :V] = np.asarray(w2, np.float32)
        w[:, V] = np.asarray(w2, np.float64).sum(axis=1)
        bp = np.zeros((VP,), np.float32)
        bp[:V] = np.asarray(b2, np.float32)
        bp[V] = np.asarray(b2, np.float64).sum()
        return _bf(w.reshape(HBLK, 128, VP)), bp

    w2l, b2l = w2pad(lin_W2, lin_b2)
    w2o, b2o = w2pad(loc_W2, loc_b2)
    use_b2 = bool(np.any(np.asarray(lin_b2)) or np.any(np.asarray(loc_b2)))
    return dict(
        inp=_bf(np.asarray(in_proj, np.float32).reshape(2, 128, M)),
        w1l=_bf(np.asarray(lin_W1, np.float32).reshape(4, 128, H)),
        w1o=_bf(np.asarray(loc_W1, np.float32).reshape(16, 128, H)),
        b1l=np.ascontiguousarray(
            np.asarray(lin_b1, np.float32).reshape(HBLK, 128, 1)),
        b1o=np.ascontiguousarray(
            np.asarray(loc_b1, np.float32).reshape(HBLK, 128, 1)),
        w2l=w2l, w2o=w2o,
        b2=_bf(np.stack([b2l, b2o]).reshape(2, 1, VP)),
        use_b2=use_b2,
    )


def prepare_inputs(chars, emb):
    """Host side: embedding gather + transpose/pad to xt, one-hot row masks."""
    x = np.asarray(emb, np.float32)[np.asarray(chars).astype(np.int64)]  # [B,T,E]
    xt = np.zeros((2, 128, CL), np.float32)
    for b in range(B):
        xTb = x[b].T  # [E, T]
        for e in range(2):
            xt[e, :, b * LPAD + W - 1:(b + 1) * LPAD] = \
                xTb[e * 128:(e + 1) * 128, :]
    xt_bf = np.ascontiguousarray(xt.astype(ml_dtypes.bfloat16))
    in_maps = []
    for c in range(N_CORES):
        msk = np.zeros((128, N_CORES), np.float32)
        msk[:, c] = 1.0
        in_maps.append(dict(xt=xt_bf, msk=msk))
    return in_maps


def assemble_output(results):
    rows = np.concatenate(
        [np.asarray(results[c]["out"], np.float32).reshape(RPC, V)
         for c in range(N_CORES)], axis=0)
    return np.ascontiguousarray(rows.reshape(B, T, V))


_CACHE = {}


def _get_program(decays, gate_W, gate_b, in_proj, lin_W1, lin_b1, lin_W2,
                 lin_b2, loc_W1, loc_b1, loc_W2, loc_b2):
    key = tuple(
        hash(np.ascontiguousarray(np.asarray(a)).tobytes())
        for a in (decays, gate_W, gate_b, in_proj, lin_W1, lin_b1, lin_W2,
                  lin_b2, loc_W1, loc_b1, loc_W2, loc_b2)
    )
    if key not in _CACHE:
        weights = _pack_weights(in_proj, lin_W1, lin_b1, lin_W2, lin_b2,
                                loc_W1, loc_b1, loc_W2, loc_b2)
        _CACHE[key] = build_program(
            np.asarray(decays, np.float32),
            np.asarray(gate_W, np.float64).reshape(-1),
            float(np.asarray(gate_b).reshape(-1)[0]),
            weights,
        )
    return _CACHE[key]


def get_nc_and_inmaps(inputs):
    nc = _get_program(
        inputs["decays"], inputs["gate_W"], inputs["gate_b"],
        inputs["in_proj"], inputs["lin_W1"], inputs["lin_b1"],
        inputs["lin_W2"], inputs["lin_b2"], inputs["loc_W1"],
        inputs["loc_b1"], inputs["loc_W2"], inputs["loc_b2"],
    )
    return nc, prepare_inputs(inputs["chars"], inputs["emb"])


def kernel(chars, emb, in_proj, decays, lin_W1, lin_b1, lin_W2, lin_b2,
           loc_W1, loc_b1, loc_W2, loc_b2, gate_W, gate_b):
    nc, in_maps = get_nc_and_inmaps(dict(
        chars=chars, emb=emb, in_proj=in_proj, decays=decays,
        lin_W1=lin_W1, lin_b1=lin_b1, lin_W2=lin_W2, lin_b2=lin_b2,
        loc_W1=loc_W1, loc_b1=loc_b1, loc_W2=loc_W2, loc_b2=loc_b2,
        gate_W=gate_W, gate_b=gate_b,
    ))
    res = run_bass_kernel_spmd(nc, in_maps, CORE_IDS)
    return assemble_output(res.results)


# revision 10
# speedup vs baseline: 4.2826x; 4.2826x over previous
"""Trainium2 Bass kernel for nn_CausalBankModel (V=32000, E=256, M=256, T=1024,
B=2, H=1024, W=8) on 8 NeuronCores.

The per-dispatch cost in this environment is dominated by shipping
ExternalInput/ExternalOutput bytes through the PJRT tunnel, so the design
minimizes per-dispatch I/O:

- All weights (in_proj, W1s, W2s, biases, decay-scan constants) are baked into
  the NEFF as inline Const tensors -> loaded to device HBM once at model load,
  zero per-dispatch cost. One shared SPMD program for all 8 cores.
- The embedding gather x = emb[chars] happens on host; the kernel ships only
  the transposed padded activations xt (bf16, ~1 MB), identical to all cores.
- Row sharding: core c computes rows [c*256, (c+1)*256) of the B*T=2048 rows
  through both MLP paths over the FULL vocab, so logit statistics (entropy,
  max, variance) are core-local: no collectives at all. The causal decay scan
  (cheap) is computed redundantly on every core from the full xt.
- Core selection is data-driven, not program-driven: each core receives a tiny
  one-hot mask [128, 8]; "my 256 rows" of states/x are extracted with 8 masked
  multiply-accumulate ops (exact for 0/1 masks), keeping one static program.
- Output is the core's [256, 32000] mixed logits in f16 (16.4 MB vs 32.8 f32).

Scan: within a 128-step chunk, states = diag(d^i) @ TriU @ diag(d^-j) via one
128x128 triangular matmul per chunk per 128-mode half, plus a per-partition
carry add (exact in f32; worst-case rescale 0.85^-127 ~ 8.8e8 is inside f32
range). Stats use ScalarE activation accum_out (fused free-axis sum); max uses
DVE reduce_max. Sum-of-logits comes from an extra W2 column holding row sums.
"""

import sys

import numpy as np

sys.path.insert(0, "/opt/trn_rl_repo")

import ml_dtypes  # noqa: E402

from concourse import bacc, mybir, tile  # noqa: E402
from concourse.bass_utils import run_bass_kernel_spmd  # noqa: E402

F32 = mybir.dt.float32
BF16 = mybir.dt.bfloat16
F16 = mybir.dt.float16
AF = mybir.ActivationFunctionType
ALU = mybir.AluOpType
X_AXIS = mybir.AxisListType.X

V, E, M, T, B, H, W = 32000, 256, 256, 1024, 2, 1024, 8
N_CORES = 8
CORE_IDS = list(range(N_CORES))
NR = B * T              # 2048 rows
RPC = NR // N_CORES     # 256 rows per core
HBLK = H // 128         # 8 hidden blocks
LPAD = T + W - 1        # 1031 padded columns per batch in x_T
CL = B * LPAD           # 2062 total xt columns
CHUNK = 128             # scan chunk length
NCH = T // CHUNK        # 8 chunks per batch
VP = 32256              # padded vocab (63 x 512); col 32000 = row-sum column
VBW = 1536              # vocab cols per streamed W2 block (3 psum chunks)
NVB = VP // VBW         # 21 blocks
VLAST = V - (NVB - 1) * VBW  # 1280 real cols in the last block
XW = W - 1 + RPC        # 263 xt cols a core needs (7 history + 256)


def _bf(a):
    return np.ascontiguousarray(np.asarray(a, np.float32).astype(ml_dtypes.bfloat16))


def _cbase(c):
    b, q = divmod(c, N_CORES // B)
    return b * LPAD + q * RPC


def build_program(decays_np, gate_w, gate_b, weights):
    """Build the per-core Bass program. ALL weights are baked into the NEFF
    as inline constants; only xt + the row mask arrive per dispatch."""
    import os
    kvar = os.environ.get("KVAR", "full")
    out_dt = {"f32out": F32, "bf16out": BF16}.get(kvar, F16)
    nc = bacc.Bacc(None, target_bir_lowering=False)

    xt_d = nc.dram_tensor("xt", [2, 128, CL], BF16, kind="ExternalInput")
    msk_d = nc.dram_tensor("msk", [128, N_CORES], F32, kind="ExternalInput")
    out_d = nc.dram_tensor("out", [2, 128, V], out_dt, kind="ExternalOutput")

    use_b2 = weights["use_b2"]

    # host-precomputed scan constants
    j = np.arange(CHUNK)
    d64 = np.asarray(decays_np, dtype=np.float64)
    tri = np.triu(np.ones((CHUNK, CHUNK), np.float32))           # [j, i] j<=i
    dneg = (d64[None, :] ** (-j[:, None])).astype(np.float32)    # [j=128, M]
    dpow = (d64[:, None] ** j[None, :]).astype(np.float32)       # [M, i=128]
    tri_t = nc.inline_tensor(tri, "tri")
    dneg_t = nc.inline_tensor(dneg, "dneg")
    dpow_t = nc.inline_tensor(dpow, "dpow")
    dvec_t = nc.inline_tensor(
        np.ascontiguousarray(d64.astype(np.float32).reshape(2, 128, 1)), "dvec"
    )
    inp_t = nc.inline_tensor(weights["inp"], "inp")      # [2,128,M] bf16
    w1l_t = nc.inline_tensor(weights["w1l"], "w1l")      # [4,128,H] bf16
    w1o_t = nc.inline_tensor(weights["w1o"], "w1o")      # [16,128,H] bf16
    b1l_t = nc.inline_tensor(weights["b1l"], "b1l")      # [HBLK,128,1] f32
    b1o_t = nc.inline_tensor(weights["b1o"], "b1o")
    w2l_t = nc.inline_tensor(weights["w2l"], "w2l")      # [HBLK,128,VP] bf16
    w2o_t = nc.inline_tensor(weights["w2o"], "w2o")
    b2_t = nc.inline_tensor(weights["b2"], "b2") if use_b2 else None

    with tile.TileContext(nc) as tc, tile.ExitStack() as top:
        sb = top.enter_context(tc.tile_pool(name="sb", bufs=1))
        dr = top.enter_context(tc.tile_pool(name="dr", bufs=1, space="DRAM"))

        # ---------- resident sbuf state ----------
        xtbf = [sb.tile([128, CL], BF16, tag=f"xtbf{e}", name=f"xtbf{e}")
                for e in range(2)]
        st_bf = [sb.tile([128, NR], BF16, tag=f"stbf{m}", name=f"stbf{m}")
                 for m in range(2)]
        msk_s = sb.tile([128, N_CORES], F32, tag="msk")
        xt_my = [sb.tile([128, XW], BF16, tag=f"xtmy{e}", name=f"xtmy{e}")
                 for e in range(2)]
        st_my = [sb.tile([128, RPC], BF16, tag=f"stmy{m}", name=f"stmy{m}")
                 for m in range(2)]
        ht = [sb.tile([128, HBLK, RPC], BF16, tag=f"ht{p}", name=f"ht{p}")
              for p in range(2)]
        ones_s = sb.tile([1, 128], BF16, tag="ones1")
        if use_b2:
            nc.vector.memset(ones_s[:], 1.0)
        stats = {}
        for p in range(2):
            for nm in ("se", "sa", "sq", "mx"):
                stats[(p, nm)] = sb.tile([128, 2, NVB], F32, tag=f"st{p}{nm}",
                                         name=f"st{p}{nm}")
            stats[(p, "sl")] = sb.tile([128, 2], F32, tag=f"st{p}sl",
                                       name=f"st{p}sl")
        ldram = [dr.tile([2, 128, VP], BF16, name=f"ldram{p}") for p in range(2)]

        for e in range(2):
            nc.sync.dma_start(out=xtbf[e][:], in_=xt_d[e])
        nc.sync.dma_start(out=msk_s[:], in_=msk_d[:])

        # ---------- causal decay scan over all rows (redundant per core) ----
        scn_cm = tile.ExitStack()
        scn = scn_cm.enter_context(tc.tile_pool(name="scn", bufs=1))
        tri_s = scn.tile([128, 128], F32, tag="tri")
        dneg_s = scn.tile([128, M], F32, tag="dneg")
        dpow_s = [scn.tile([128, 128], F32, tag=f"dpow{m}", name=f"dpow{m}")
                  for m in range(2)]
        dvec_s = [scn.tile([128, 1], F32, tag=f"dvec{m}", name=f"dvec{m}")
                  for m in range(2)]
        inp_s = scn.tile([128, 2, M], BF16, tag="inp")
        nc.sync.dma_start(out=tri_s[:], in_=tri_t[:])
        nc.sync.dma_start(out=dneg_s[:], in_=dneg_t[:])
        for m in range(2):
            nc.sync.dma_start(out=dpow_s[m][:], in_=dpow_t[m * 128:(m + 1) * 128, :])
            nc.sync.dma_start(out=dvec_s[m][:], in_=dvec_t[m])
            nc.sync.dma_start(out=inp_s[:, m, :], in_=inp_t[m])

        with (
            tc.tile_pool(name="cv", bufs=4) as cv,
            tc.tile_pool(name="pd", bufs=2, space="PSUM") as pd,
            tc.tile_pool(name="pc", bufs=4, space="PSUM") as pc,
        ):
            carry = {}
            for b in range(B):
                for m in range(2):
                    cz = cv.tile([128, 1], F32, tag=f"car{b}{m}")
                    nc.vector.memset(cz[:], 0.0)
                    carry[(b, m)] = cz
            for c in range(NCH):
                for b in range(B):
                    col = b * LPAD + (W - 1) + c * CHUNK
                    psd = pd.tile([128, M], F32, tag="psd")
                    for e in range(2):
                        nc.tensor.matmul(
                            psd[:], xtbf[e][:, col:col + 128], inp_s[:, e, :],
                            start=(e == 0), stop=(e == 1),
                        )
                    scaled = cv.tile([128, M], F32, tag="scaled")
                    nc.vector.tensor_mul(scaled[:], psd[:], dneg_s[:])
                    n0 = b * T + c * CHUNK
                    for m in range(2):
                        psc = pc.tile([128, 128], F32, tag="psc")
                        nc.tensor.matmul(
                            psc[:], scaled[:, m * 128:(m + 1) * 128], tri_s[:],
                            start=True, stop=True,
                        )
                        nc.vector.tensor_scalar_add(psc[:], psc[:], carry[(b, m)][:])
                        nc.vector.tensor_mul(
                            st_bf[m][:, n0:n0 + CHUNK], psc[:], dpow_s[m][:]
                        )
                        cn = cv.tile([128, 1], F32, tag=f"car{b}{m}")
                        nc.vector.tensor_mul(
                            cn[:], psc[:, 127:128], dpow_s[m][:, 127:128]
                        )
                        nc.vector.tensor_mul(cn[:], cn[:], dvec_s[m][:])
                        carry[(b, m)] = cn
        scn_cm.close()

        # ---------- masked row-select: my 256 rows (data-driven, no branches)
        for e in range(2):
            nc.vector.tensor_scalar_mul(
                xt_my[e][:], xtbf[e][:, _cbase(0):_cbase(0) + XW], msk_s[:, 0:1]
            )
            for c in range(1, N_CORES):
                nc.vector.scalar_tensor_tensor(
                    out=xt_my[e][:], in0=xtbf[e][:, _cbase(c):_cbase(c) + XW],
                    scalar=msk_s[:, c:c + 1], in1=xt_my[e][:],
                    op0=ALU.mult, op1=ALU.add,
                )
        for m in range(2):
            nc.vector.tensor_scalar_mul(
                st_my[m][:], st_bf[m][:, 0:RPC], msk_s[:, 0:1]
            )
            for c in range(1, N_CORES):
                nc.vector.scalar_tensor_tensor(
                    out=st_my[m][:], in0=st_bf[m][:, c * RPC:(c + 1) * RPC],
                    scalar=msk_s[:, c:c + 1], in1=st_my[m][:],
                    op0=ALU.mult, op1=ALU.add,
                )

        # ---------- W1 (hidden layer) for my rows, both paths ----------
        for path in range(2):
            w1_t, b1_t = (w1l_t, b1l_t) if path == 0 else (w1o_t, b1o_t)
            nk1 = 4 if path == 0 else 16

            def rhs_for(kk):
                if path == 0:
                    if kk < 2:
                        return st_my[kk][:]
                    return xt_my[kk - 2][:, W - 1:W - 1 + RPC]
                o, e = divmod(kk, 2)
                return xt_my[e][:, W - 1 - o:W - 1 - o + RPC]

            with (
                tc.tile_pool(name=f"w1p{path}", bufs=1) as w1p,
                tc.tile_pool(name=f"psh{path}", bufs=2, space="PSUM") as psh,
            ):
                w1_s = w1p.tile([128, nk1, H], BF16, tag="w1")
                for kk in range(nk1):
                    nc.sync.dma_start(out=w1_s[:, kk, :], in_=w1_t[kk])
                b1_s = w1p.tile([128, HBLK], F32, tag="b1")
                for hh in range(HBLK):
                    nc.sync.dma_start(out=b1_s[:, hh:hh + 1], in_=b1_t[hh])
                for hh in range(HBLK):
                    ps = psh.tile([128, RPC], F32, tag="ph", bufs=2)
                    for kk in range(nk1):
                        nc.tensor.matmul(
                            ps[:], w1_s[:, kk, hh * 128:(hh + 1) * 128],
                            rhs_for(kk), start=(kk == 0), stop=(kk == nk1 - 1),
                        )
                    nc.scalar.activation(
                        ht[path][:, hh, :], ps[:], AF.Relu,
                        bias=b1_s[:, hh:hh + 1],
                    )

        # ---------- W2 stream + logit stats, both paths ----------
        if kvar == "nostats":
            for p in range(2):
                for nm in ("se", "sa", "sq", "mx", "sl"):
                    nc.vector.memset(stats[(p, nm)][:], 1.0)
        if kvar in ("now2", "noscan"):
            done = sb.tile([128, 16], out_dt, tag="done")
            nc.vector.memset(done[:], 1.0)
            nc.vector.tensor_scalar_add(done[:, :1], done[:, :1],
                                        st_my[0][:, :1])
            nc.vector.tensor_scalar_add(done[:, 1:2], done[:, 1:2],
                                        ht[0][:, 0, :1])
            nc.sync.dma_start(out=out_d[0, :, 0:16], in_=done[:])
            nc.compile()
            return nc
        for path in range(2):
            w2_t = w2l_t if path == 0 else w2o_t
            s_se, s_sa = stats[(path, "se")], stats[(path, "sa")]
            s_sq, s_mx = stats[(path, "sq")], stats[(path, "mx")]
            s_sl = stats[(path, "sl")]
            ld = ldram[path]
            with (
                tc.tile_pool(name=f"w2p{path}", bufs=2) as w2p,
                tc.tile_pool(name=f"stp{path}", bufs=2) as stp,
                tc.tile_pool(name=f"pw{path}", bufs=2, space="PSUM") as pw,
            ):
                if use_b2:
                    b2_s = w2p.tile([1, VP], BF16, tag="b2", bufs=1)
                    nc.sync.dma_start(out=b2_s[:], in_=b2_t[path])
                for vb in range(NVB):
                    w2s = w2p.tile([128, HBLK, VBW], BF16, tag="w2s", bufs=2)
                    for hh in range(HBLK):
                        nc.sync.dma_start(
                            out=w2s[:, hh, :],
                            in_=w2_t[hh, :, vb * VBW:(vb + 1) * VBW],
                        )
                    w = VBW if vb < NVB - 1 else VLAST
                    for rb in range(2):
                        stage = stp.tile([128, VBW], BF16, tag="stage", bufs=3)
                        for ch in range(3):
                            psl = pw.tile([128, 512], F32, tag=f"pl{ch}",
                                          name=f"pl{ch}", bufs=2)
                            for hh in range(HBLK):
                                nc.tensor.matmul(
                                    psl[:],
                                    ht[path][:, hh, rb * 128:(rb + 1) * 128],
                                    w2s[:, hh, ch * 512:(ch + 1) * 512],
                                    start=(hh == 0),
                                    stop=(hh == HBLK - 1) and not use_b2,
                                )
                            if use_b2:
                                nc.tensor.matmul(
                                    psl[:], ones_s[:],
                                    b2_s[:, vb * VBW + ch * 512:
                                         vb * VBW + (ch + 1) * 512],
                                    start=False, stop=True,
                                )
                            nc.vector.tensor_copy(
                                stage[:, ch * 512:(ch + 1) * 512], psl[:]
                            )
                        if kvar != "nostats":
                            # ScalarE runs ONLY Exp (no accum_out, no act
                            # table switches); every reduction goes to DVE.
                            et = stp.tile([128, VBW], BF16, tag="et", bufs=2)
                            nc.scalar.activation(
                                et[:, :w], stage[:, :w], AF.Exp,
                            )
                            nc.vector.tensor_reduce(
                                s_se[:, rb, vb:vb + 1], et[:, :w],
                                axis=X_AXIS, op=ALU.add,
                            )
                            le = stp.tile([128, VBW], BF16, tag="le", bufs=2)
                            nc.vector.tensor_mul(le[:, :w], stage[:, :w],
                                                 et[:, :w])
                            nc.vector.tensor_reduce(
                                s_sa[:, rb, vb:vb + 1], le[:, :w],
                                axis=X_AXIS, op=ALU.add,
                            )
                            sqt = stp.tile([128, VBW], BF16, tag="sqt", bufs=2)
                            nc.vector.tensor_mul(sqt[:, :w], stage[:, :w],
                                                 stage[:, :w])
                            nc.vector.tensor_reduce(
                                s_sq[:, rb, vb:vb + 1], sqt[:, :w],
                                axis=X_AXIS, op=ALU.add,
                            )
                            nc.vector.tensor_reduce(
                                s_mx[:, rb, vb:vb + 1], stage[:, :w],
                                axis=X_AXIS, op=ALU.max,
                            )
                            if vb == NVB - 1:
                                nc.vector.tensor_copy(
                                    s_sl[:, rb:rb + 1],
                                    stage[:, VLAST:VLAST + 1]
                                )
                        nc.sync.dma_start(
                            out=ld[rb, :, vb * VBW:(vb + 1) * VBW], in_=stage[:]
                        )

        # ---------- gate from local stats ----------
        gate = sb.tile([128, 2], F32, tag="gate")
        with tc.tile_pool(name="gtp", bufs=1) as gtp:
            feats = []
            for p in range(2):
                Sg = gtp.tile([128, 2], F32, tag=f"Sg{p}")
                Ag = gtp.tile([128, 2], F32, tag=f"Ag{p}")
                Qg = gtp.tile([128, 2], F32, tag=f"Qg{p}")
                Mg = gtp.tile([128, 2], F32, tag=f"Mg{p}")
                for rb in range(2):
                    nc.vector.tensor_reduce(
                        Sg[:, rb:rb + 1], stats[(p, "se")][:, rb, :],
                        axis=X_AXIS, op=ALU.add)
                    nc.vector.tensor_reduce(
                        Ag[:, rb:rb + 1], stats[(p, "sa")][:, rb, :],
                        axis=X_AXIS, op=ALU.add)
                    nc.vector.tensor_reduce(
                        Qg[:, rb:rb + 1], stats[(p, "sq")][:, rb, :],
                        axis=X_AXIS, op=ALU.add)
                    nc.vector.tensor_reduce(
                        Mg[:, rb:rb + 1], stats[(p, "mx")][:, rb, :],
                        axis=X_AXIS, op=ALU.max)
                Lg = stats[(p, "sl")]
                rS = gtp.tile([128, 2], F32, tag=f"rS{p}")
                nc.vector.reciprocal(rS[:], Sg[:])
                AoS = gtp.tile([128, 2], F32, tag=f"AoS{p}")
                nc.vector.tensor_mul(AoS[:], Ag[:], rS[:])
                lnS = gtp.tile([128, 2], F32, tag=f"lnS{p}")
                nc.scalar.activation(lnS[:], Sg[:], AF.Ln)
                ent = gtp.tile([128, 2], F32, tag=f"ent{p}")
                nc.vector.tensor_sub(ent[:], lnS[:], AoS[:])
                mean = gtp.tile([128, 2], F32, tag=f"mean{p}")
                nc.vector.tensor_scalar_mul(mean[:], Lg[:], 1.0 / V)
                m2 = gtp.tile([128, 2], F32, tag=f"m2{p}")
                nc.vector.tensor_mul(m2[:], mean[:], mean[:])
                var = gtp.tile([128, 2], F32, tag=f"var{p}")
                nc.vector.tensor_scalar_mul(var[:], Qg[:], 1.0 / V)
                nc.vector.tensor_sub(var[:], var[:], m2[:])
                feats += [ent, Mg, var]

            acc = gtp.tile([128, 2], F32, tag="gacc")
            nc.vector.tensor_scalar_mul(acc[:], feats[0][:], float(gate_w[0]))
            for i in range(1, 6):
                nc.vector.scalar_tensor_tensor(
                    out=acc[:], in0=feats[i][:], scalar=float(gate_w[i]),
                    in1=acc[:], op0=ALU.mult, op1=ALU.add,
                )
            nc.scalar.activation(
                gate[:], acc[:], AF.Sigmoid, bias=float(gate_b), scale=1.0
            )

        # ---------- gated mix -> f16 output ----------
        with tc.tile_pool(name="mx", bufs=3) as mxp:
            for rb in range(2):
                for vb in range(NVB):
                    w = VBW if vb < NVB - 1 else VLAST
                    lin_s = mxp.tile([128, VBW], BF16, tag="lin")
                    loc_s = mxp.tile([128, VBW], BF16, tag="loc")
                    nc.sync.dma_start(
                        out=lin_s[:, :w], in_=ldram[0][rb, :, vb * VBW:vb * VBW + w])
                    nc.sync.dma_start(
                        out=loc_s[:, :w], in_=ldram[1][rb, :, vb * VBW:vb * VBW + w])
                    d = mxp.tile([128, VBW], BF16, tag="d")
                    nc.vector.tensor_sub(d[:, :w], lin_s[:, :w], loc_s[:, :w])
                    o = mxp.tile([128, VBW], out_dt, tag="o")
                    nc.vector.scalar_tensor_tensor(
                        out=o[:, :w], in0=d[:, :w], scalar=gate[:, rb:rb + 1],
                        in1=loc_s[:, :w], op0=ALU.mult, op1=ALU.add,
                    )
                    if kvar == "noout":
                        if rb == 0 and vb == 0:
                            nc.sync.dma_start(out=out_d[0, :, 0:16],
                                              in_=o[:, :16])
                    else:
                        nc.sync.dma_start(
                            out=out_d[rb, :, vb * VBW:vb * VBW + w],
                            in_=o[:, :w]
                        )

    nc.compile()
    return nc


def _pack_weights(in_proj, lin_W1, lin_b1, lin_W2, lin_b2,
                  loc_W1, loc_b1, loc_W2, loc_b2):
    def w2pad(w2, b2):
        w = np.zeros((H, VP), np.float32)
        w[:, :V] = np.asarray(w2, np.float32)
        w[:, V] = np.asarray(w2, np.float64).sum(axis=1)
        bp = np.zeros((VP,), np.float32)
        bp[:V] = np.asarray(b2, np.float32)
        bp[V] = np.asarray(b2, np.float64).sum()
        return _bf(w.reshape(HBLK, 128, VP)), bp

    w2l, b2l = w2pad(lin_W2, lin_b2)
    w2o, b2o = w2pad(loc_W2, loc_b2)
    use_b2 = bool(np.any(np.asarray(lin_b2)) or np.any(np.asarray(loc_b2)))
    return dict(
        inp=_bf(np.asarray(in_proj, np.float32).reshape(2, 128, M)),
        w1l=_bf(np.asarray(lin_W1, np.float32).reshape(4, 128, H)),
        w1o=_bf(np.asarray(loc_W1, np.float32).reshape(16, 128, H)),
        b1l=np.ascontiguousarray(
            np.asarray(lin_b1, np.float32).reshape(HBLK, 128, 1)),
        b1o=np.ascontiguousarray(
            np.asarray(loc_b1, np.float32).reshape(HBLK, 128, 1)),
        w2l=w2l, w2o=w2o,
        b2=_bf(np.stack([b2l, b2o]).reshape(2, 1, VP)),
        use_b2=use_b2,
    )


def prepare_inputs(chars, emb):
    """Host side: embedding gather + transpose/pad to xt, one-hot row masks."""
    x = np.asarray(emb, np.float32)[np.asarray(chars).astype(np.int64)]  # [B,T,E]
    xt = np.zeros((2, 128, CL), np.float32)
    for b in range(B):
        xTb = x[b].T  # [E, T]
        for e in range(2):
            xt[e, :, b * LPAD + W - 1:(b + 1) * LPAD] = \
                xTb[e * 128:(e + 1) * 128, :]
    xt_bf = np.ascontiguousarray(xt.astype(ml_dtypes.bfloat16))
    in_maps = []
    for c in range(N_CORES):
        msk = np.zeros((128, N_CORES), np.float32)
        msk[:, c] = 1.0
        in_maps.append(dict(xt=xt_bf, msk=msk))
    return in_maps


def assemble_output(results):
    rows = np.concatenate(
        [np.asarray(results[c]["out"], np.float32).reshape(RPC, V)
         for c in range(N_CORES)], axis=0)
    return np.ascontiguousarray(rows.reshape(B, T, V))


_CACHE = {}


def _get_program(decays, gate_W, gate_b, in_proj, lin_W1, lin_b1, lin_W2,
                 lin_b2, loc_W1, loc_b1, loc_W2, loc_b2):
    import os
    key = (os.environ.get("KVAR", "full"),) + tuple(
        hash(np.ascontiguousarray(np.asarray(a)).tobytes())
        for a in (decays, gate_W, gate_b, in_proj, lin_W1, lin_b1, lin_W2,
                  lin_b2, loc_W1, loc_b1, loc_W2, loc_b2)
    )
    if key not in _CACHE:
        weights = _pack_weights(in_proj, lin_W1, lin_b1, lin_W2, lin_b2,
                                loc_W1, loc_b1, loc_W2, loc_b2)
        _CACHE[key] = build_program(
            np.asarray(decays, np.float32),
            np.asarray(gate_W, np.float64).reshape(-1),
            float(np.asarray(gate_b).reshape(-1)[0]),
            weights,
        )
    return _CACHE[key]


def get_nc_and_inmaps(inputs):
    nc = _get_program(
        inputs["decays"], inputs["gate_W"], inputs["gate_b"],
        inputs["in_proj"], inputs["lin_W1"], inputs["lin_b1"],
        inputs["lin_W2"], inputs["lin_b2"], inputs["loc_W1"],
        inputs["loc_b1"], inputs["loc_W2"], inputs["loc_b2"],
    )
    return nc, prepare_inputs(inputs["chars"], inputs["emb"])


def kernel(chars, emb, in_proj, decays, lin_W1, lin_b1, lin_W2, lin_b2,
           loc_W1, loc_b1, loc_W2, loc_b2, gate_W, gate_b):
    nc, in_maps = get_nc_and_inmaps(dict(
        chars=chars, emb=emb, in_proj=in_proj, decays=decays,
        lin_W1=lin_W1, lin_b1=lin_b1, lin_W2=lin_W2, lin_b2=lin_b2,
        loc_W1=loc_W1, loc_b1=loc_b1, loc_W2=loc_W2, loc_b2=loc_b2,
        gate_W=gate_W, gate_b=gate_b,
    ))
    res = run_bass_kernel_spmd(nc, in_maps, CORE_IDS)
    return assemble_output(res.results)


# revision 17
# speedup vs baseline: 6.8450x; 1.5983x over previous
"""Trainium2 Bass kernel for nn_CausalBankModel (V=32000, E=256, M=256, T=1024,
B=2, H=1024, W=8) on 8 NeuronCores.

The per-dispatch cost in this environment is dominated by shipping
ExternalInput/ExternalOutput bytes through the PJRT tunnel, so the design
minimizes per-dispatch I/O:

- All weights (in_proj, W1s, W2s, biases, decay-scan constants) are baked into
  the NEFF as inline Const tensors -> loaded to device HBM once at model load,
  zero per-dispatch cost. One shared SPMD program for all 8 cores.
- The embedding gather x = emb[chars] happens on host; the kernel ships only
  the transposed padded activations xt (bf16, ~1 MB), identical to all cores.
- Row sharding: core c computes rows [c*256, (c+1)*256) of the B*T=2048 rows
  through both MLP paths over the FULL vocab, so logit statistics (entropy,
  max, variance) are core-local: no collectives at all. The causal decay scan
  (cheap) is computed redundantly on every core from the full xt.
- Core selection is data-driven, not program-driven: each core receives a tiny
  one-hot mask [128, 8]; "my 256 rows" of states/x are extracted with 8 masked
  multiply-accumulate ops (exact for 0/1 masks), keeping one static program.
- Output is the core's [256, 32000] mixed logits in f16 (16.4 MB vs 32.8 f32).

Scan: within a 128-step chunk, states = diag(d^i) @ TriU @ diag(d^-j) via one
128x128 triangular matmul per chunk per 128-mode half, plus a per-partition
carry add (exact in f32; worst-case rescale 0.85^-127 ~ 8.8e8 is inside f32
range). Stats use ScalarE activation accum_out (fused free-axis sum); max uses
DVE reduce_max. Sum-of-logits comes from an extra W2 column holding row sums.
"""

import sys

import numpy as np

sys.path.insert(0, "/opt/trn_rl_repo")

import ml_dtypes  # noqa: E402

from concourse import bacc, mybir, tile  # noqa: E402
from concourse.bass_utils import run_bass_kernel_spmd  # noqa: E402

F32 = mybir.dt.float32
BF16 = mybir.dt.bfloat16
F16 = mybir.dt.float16
AF = mybir.ActivationFunctionType
ALU = mybir.AluOpType
X_AXIS = mybir.AxisListType.X

V, E, M, T, B, H, W = 32000, 256, 256, 1024, 2, 1024, 8
N_CORES = 8
CORE_IDS = list(range(N_CORES))
NR = B * T              # 2048 rows
RPC = NR // N_CORES     # 256 rows per core
HBLK = H // 128         # 8 hidden blocks
LPAD = T + W - 1        # 1031 padded columns per batch in x_T
CL = B * LPAD           # 2062 total xt columns
CHUNK = 128             # scan chunk length
NCH = T // CHUNK        # 8 chunks per batch
VP = 32256              # padded vocab (63 x 512); col 32000 = row-sum column
VBW = 1536              # vocab cols per streamed W2 block (3 psum chunks)
NVB = VP // VBW         # 21 blocks
VLAST = V - (NVB - 1) * VBW  # 1280 real cols in the last block
XW = W - 1 + RPC        # 263 xt cols a core needs (7 history + 256)


def _bf(a):
    return np.ascontiguousarray(np.asarray(a, np.float32).astype(ml_dtypes.bfloat16))


def _cbase(c):
    b, q = divmod(c, N_CORES // B)
    return b * LPAD + q * RPC


def build_program(decays_np, gate_w, gate_b, weights):
    """Build the per-core Bass program. ALL weights are baked into the NEFF
    as inline constants; only xt + the row mask arrive per dispatch."""
    import os
    kvar = os.environ.get("KVAR", "full")
    i8 = kvar == "i8out"
    out_dt = {"f32out": F32, "bf16out": BF16,
              "i8out": mybir.dt.int8}.get(kvar, F16)
    nc = bacc.Bacc(None, target_bir_lowering=False)

    xt_d = nc.dram_tensor("xt", [2, 128, CL], BF16, kind="ExternalInput")
    msk_d = nc.dram_tensor("msk", [128, N_CORES], F32, kind="ExternalInput")
    out_d = nc.dram_tensor("out", [2, 128, V], out_dt, kind="ExternalOutput")
    sc_d = (nc.dram_tensor("sc", [128, 2], F32, kind="ExternalOutput")
            if i8 else None)

    use_b2 = weights["use_b2"]

    # host-precomputed scan constants
    j = np.arange(CHUNK)
    d64 = np.asarray(decays_np, dtype=np.float64)
    tri = np.triu(np.ones((CHUNK, CHUNK), np.float32))           # [j, i] j<=i
    dneg = (d64[None, :] ** (-j[:, None])).astype(np.float32)    # [j=128, M]
    dpow = (d64[:, None] ** j[None, :]).astype(np.float32)       # [M, i=128]
    tri_t = nc.inline_tensor(tri, "tri")
    dneg_t = nc.inline_tensor(dneg, "dneg")
    dpow_t = nc.inline_tensor(dpow, "dpow")
    dvec_t = nc.inline_tensor(
        np.ascontiguousarray(d64.astype(np.float32).reshape(2, 128, 1)), "dvec"
    )
    inp_t = nc.inline_tensor(weights["inp"], "inp")      # [2,128,M] bf16
    w1l_t = nc.inline_tensor(weights["w1l"], "w1l")      # [4,128,H] bf16
    w1o_t = nc.inline_tensor(weights["w1o"], "w1o")      # [16,128,H] bf16
    b1l_t = nc.inline_tensor(weights["b1l"], "b1l")      # [HBLK,128,1] f32
    b1o_t = nc.inline_tensor(weights["b1o"], "b1o")
    w2l_t = nc.inline_tensor(weights["w2l"], "w2l")      # [HBLK,128,VP] bf16
    w2o_t = nc.inline_tensor(weights["w2o"], "w2o")
    b2_t = nc.inline_tensor(weights["b2"], "b2") if use_b2 else None

    with tile.TileContext(nc) as tc, tile.ExitStack() as top:
        sb = top.enter_context(tc.tile_pool(name="sb", bufs=1))
        dr = top.enter_context(tc.tile_pool(name="dr", bufs=1, space="DRAM"))

        # ---------- resident sbuf state ----------
        xtbf = [sb.tile([128, CL], BF16, tag=f"xtbf{e}", name=f"xtbf{e}")
                for e in range(2)]
        st_bf = [sb.tile([128, NR], BF16, tag=f"stbf{m}", name=f"stbf{m}")
                 for m in range(2)]
        msk_s = sb.tile([128, N_CORES], F32, tag="msk")
        xt_my = [sb.tile([128, XW], BF16, tag=f"xtmy{e}", name=f"xtmy{e}")
                 for e in range(2)]
        st_my = [sb.tile([128, RPC], BF16, tag=f"stmy{m}", name=f"stmy{m}")
                 for m in range(2)]
        ht = [sb.tile([128, HBLK, RPC], BF16, tag=f"ht{p}", name=f"ht{p}")
              for p in range(2)]
        ones_s = sb.tile([1, 128], BF16, tag="ones1")
        if use_b2:
            nc.vector.memset(ones_s[:], 1.0)
        stats = {}
        for p in range(2):
            for nm in (("se", "sa", "sq", "mx", "mq") if i8 else
                       ("se", "sa", "sq", "mx")):
                stats[(p, nm)] = sb.tile([128, 2, NVB], F32, tag=f"st{p}{nm}",
                                         name=f"st{p}{nm}")
            stats[(p, "sl")] = sb.tile([128, 2], F32, tag=f"st{p}sl",
                                       name=f"st{p}sl")
        ldram = [dr.tile([2, 128, VP], BF16, name=f"ldram{p}") for p in range(2)]

        for e in range(2):
            nc.sync.dma_start(out=xtbf[e][:], in_=xt_d[e])
        nc.sync.dma_start(out=msk_s[:], in_=msk_d[:])

        # ---------- causal decay scan over all rows (redundant per core) ----
        scn_cm = tile.ExitStack()
        scn = scn_cm.enter_context(tc.tile_pool(name="scn", bufs=1))
        tri_s = scn.tile([128, 128], F32, tag="tri")
        dneg_s = scn.tile([128, M], F32, tag="dneg")
        dpow_s = [scn.tile([128, 128], F32, tag=f"dpow{m}", name=f"dpow{m}")
                  for m in range(2)]
        dvec_s = [scn.tile([128, 1], F32, tag=f"dvec{m}", name=f"dvec{m}")
                  for m in range(2)]
        inp_s = scn.tile([128, 2, M], BF16, tag="inp")
        nc.sync.dma_start(out=tri_s[:], in_=tri_t[:])
        nc.sync.dma_start(out=dneg_s[:], in_=dneg_t[:])
        for m in range(2):
            nc.sync.dma_start(out=dpow_s[m][:], in_=dpow_t[m * 128:(m + 1) * 128, :])
            nc.sync.dma_start(out=dvec_s[m][:], in_=dvec_t[m])
            nc.sync.dma_start(out=inp_s[:, m, :], in_=inp_t[m])

        with (
            tc.tile_pool(name="cv", bufs=4) as cv,
            tc.tile_pool(name="pd", bufs=2, space="PSUM") as pd,
            tc.tile_pool(name="pc", bufs=4, space="PSUM") as pc,
        ):
            carry = {}
            for b in range(B):
                for m in range(2):
                    cz = cv.tile([128, 1], F32, tag=f"car{b}{m}")
                    nc.vector.memset(cz[:], 0.0)
                    carry[(b, m)] = cz
            for c in range(NCH):
                for b in range(B):
                    col = b * LPAD + (W - 1) + c * CHUNK
                    psd = pd.tile([128, M], F32, tag="psd")
                    for e in range(2):
                        nc.tensor.matmul(
                            psd[:], xtbf[e][:, col:col + 128], inp_s[:, e, :],
                            start=(e == 0), stop=(e == 1),
                        )
                    scaled = cv.tile([128, M], F32, tag="scaled")
                    nc.vector.tensor_mul(scaled[:], psd[:], dneg_s[:])
                    n0 = b * T + c * CHUNK
                    for m in range(2):
                        psc = pc.tile([128, 128], F32, tag="psc")
                        nc.tensor.matmul(
                            psc[:], scaled[:, m * 128:(m + 1) * 128], tri_s[:],
                            start=True, stop=True,
                        )
                        nc.vector.tensor_scalar_add(psc[:], psc[:], carry[(b, m)][:])
                        nc.vector.tensor_mul(
                            st_bf[m][:, n0:n0 + CHUNK], psc[:], dpow_s[m][:]
                        )
                        cn = cv.tile([128, 1], F32, tag=f"car{b}{m}")
                        nc.vector.tensor_mul(
                            cn[:], psc[:, 127:128], dpow_s[m][:, 127:128]
                        )
                        nc.vector.tensor_mul(cn[:], cn[:], dvec_s[m][:])
                        carry[(b, m)] = cn
        scn_cm.close()

        # ---------- masked row-select: my 256 rows (data-driven, no branches)
        for e in range(2):
            nc.vector.tensor_scalar_mul(
                xt_my[e][:], xtbf[e][:, _cbase(0):_cbase(0) + XW], msk_s[:, 0:1]
            )
            for c in range(1, N_CORES):
                nc.vector.scalar_tensor_tensor(
                    out=xt_my[e][:], in0=xtbf[e][:, _cbase(c):_cbase(c) + XW],
                    scalar=msk_s[:, c:c + 1], in1=xt_my[e][:],
                    op0=ALU.mult, op1=ALU.add,
                )
        for m in range(2):
            nc.vector.tensor_scalar_mul(
                st_my[m][:], st_bf[m][:, 0:RPC], msk_s[:, 0:1]
            )
            for c in range(1, N_CORES):
                nc.vector.scalar_tensor_tensor(
                    out=st_my[m][:], in0=st_bf[m][:, c * RPC:(c + 1) * RPC],
                    scalar=msk_s[:, c:c + 1], in1=st_my[m][:],
                    op0=ALU.mult, op1=ALU.add,
                )

        # ---------- W1 (hidden layer) for my rows, both paths ----------
        for path in range(2):
            w1_t, b1_t = (w1l_t, b1l_t) if path == 0 else (w1o_t, b1o_t)
            nk1 = 4 if path == 0 else 16

            def rhs_for(kk):
                if path == 0:
                    if kk < 2:
                        return st_my[kk][:]
                    return xt_my[kk - 2][:, W - 1:W - 1 + RPC]
                o, e = divmod(kk, 2)
                return xt_my[e][:, W - 1 - o:W - 1 - o + RPC]

            with (
                tc.tile_pool(name=f"w1p{path}", bufs=1) as w1p,
                tc.tile_pool(name=f"psh{path}", bufs=2, space="PSUM") as psh,
            ):
                w1_s = w1p.tile([128, nk1, H], BF16, tag="w1")
                for kk in range(nk1):
                    nc.sync.dma_start(out=w1_s[:, kk, :], in_=w1_t[kk])
                b1_s = w1p.tile([128, HBLK], F32, tag="b1")
                for hh in range(HBLK):
                    nc.sync.dma_start(out=b1_s[:, hh:hh + 1], in_=b1_t[hh])
                for hh in range(HBLK):
                    ps = psh.tile([128, RPC], F32, tag="ph", bufs=2)
                    for kk in range(nk1):
                        nc.tensor.matmul(
                            ps[:], w1_s[:, kk, hh * 128:(hh + 1) * 128],
                            rhs_for(kk), start=(kk == 0), stop=(kk == nk1 - 1),
                        )
                    nc.scalar.activation(
                        ht[path][:, hh, :], ps[:], AF.Relu,
                        bias=b1_s[:, hh:hh + 1],
                    )

        # ---------- W2 stream + logit stats, both paths ----------
        if kvar == "nostats":
            for p in range(2):
                for nm in ("se", "sa", "sq", "mx", "sl"):
                    nc.vector.memset(stats[(p, nm)][:], 1.0)
        if kvar in ("now2", "noscan"):
            done = sb.tile([128, 16], out_dt, tag="done")
            nc.vector.memset(done[:], 1.0)
            nc.vector.tensor_scalar_add(done[:, :1], done[:, :1],
                                        st_my[0][:, :1])
            nc.vector.tensor_scalar_add(done[:, 1:2], done[:, 1:2],
                                        ht[0][:, 0, :1])
            nc.sync.dma_start(out=out_d[0, :, 0:16], in_=done[:])
            nc.compile()
            return nc
        for path in range(2):
            w2_t = w2l_t if path == 0 else w2o_t
            s_se, s_sa = stats[(path, "se")], stats[(path, "sa")]
            s_sq, s_mx = stats[(path, "sq")], stats[(path, "mx")]
            s_sl = stats[(path, "sl")]
            ld = ldram[path]
            with (
                tc.tile_pool(name=f"w2p{path}", bufs=2) as w2p,
                tc.tile_pool(name=f"stp{path}", bufs=2) as stp,
                tc.tile_pool(name=f"pw{path}", bufs=2, space="PSUM") as pw,
            ):
                if use_b2:
                    b2_s = w2p.tile([1, VP], BF16, tag="b2", bufs=1)
                    nc.sync.dma_start(out=b2_s[:], in_=b2_t[path])
                for vb in range(NVB):
                    w2s = w2p.tile([128, HBLK, VBW], BF16, tag="w2s", bufs=2)
                    for hh in range(HBLK):
                        nc.sync.dma_start(
                            out=w2s[:, hh, :],
                            in_=w2_t[hh, :, vb * VBW:(vb + 1) * VBW],
                        )
                    w = VBW if vb < NVB - 1 else VLAST
                    for rb in range(2):
                        stage = stp.tile([128, VBW], BF16, tag="stage", bufs=3)
                        for ch in range(3):
                            psl = pw.tile([128, 512], F32, tag=f"pl{ch}",
                                          name=f"pl{ch}", bufs=2)
                            for hh in range(HBLK):
                                nc.tensor.matmul(
                                    psl[:],
                                    ht[path][:, hh, rb * 128:(rb + 1) * 128],
                                    w2s[:, hh, ch * 512:(ch + 1) * 512],
                                    start=(hh == 0),
                                    stop=(hh == HBLK - 1) and not use_b2,
                                )
                            if use_b2:
                                nc.tensor.matmul(
                                    psl[:], ones_s[:],
                                    b2_s[:, vb * VBW + ch * 512:
                                         vb * VBW + (ch + 1) * 512],
                                    start=False, stop=True,
                                )
                            nc.vector.tensor_copy(
                                stage[:, ch * 512:(ch + 1) * 512], psl[:]
                            )
                        if kvar != "nostats":
                            # ScalarE runs ONLY Exp (no accum_out, no act
                            # table switches); every reduction goes to DVE.
                            et = stp.tile([128, VBW], BF16, tag="et", bufs=2)
                            nc.scalar.activation(
                                et[:, :w], stage[:, :w], AF.Exp,
                            )
                            nc.vector.tensor_reduce(
                                s_se[:, rb, vb:vb + 1], et[:, :w],
                                axis=X_AXIS, op=ALU.add,
                            )
                            le = stp.tile([128, VBW], BF16, tag="le", bufs=2)
                            nc.vector.tensor_mul(le[:, :w], stage[:, :w],
                                                 et[:, :w])
                            nc.vector.tensor_reduce(
                                s_sa[:, rb, vb:vb + 1], le[:, :w],
                                axis=X_AXIS, op=ALU.add,
                            )
                            sqt = stp.tile([128, VBW], BF16, tag="sqt", bufs=2)
                            nc.vector.tensor_mul(sqt[:, :w], stage[:, :w],
                                                 stage[:, :w])
                            nc.vector.tensor_reduce(
                                s_sq[:, rb, vb:vb + 1], sqt[:, :w],
                                axis=X_AXIS, op=ALU.add,
                            )
                            if i8:
                                nc.vector.tensor_reduce(
                                    stats[(path, "mq")][:, rb, vb:vb + 1],
                                    sqt[:, :w], axis=X_AXIS, op=ALU.max,
                                )
                            nc.vector.tensor_reduce(
                                s_mx[:, rb, vb:vb + 1], stage[:, :w],
                                axis=X_AXIS, op=ALU.max,
                            )
                            if vb == NVB - 1:
                                nc.vector.tensor_copy(
                                    s_sl[:, rb:rb + 1],
                                    stage[:, VLAST:VLAST + 1]
                                )
                        nc.sync.dma_start(
                            out=ld[rb, :, vb * VBW:(vb + 1) * VBW], in_=stage[:]
                        )

        # ---------- gate from local stats ----------
        gate = sb.tile([128, 2], F32, tag="gate")
        if i8:
            bound = sb.tile([128, 2], F32, tag="bound")
            qscale = sb.tile([128, 2], F32, tag="qscale")
        with tc.tile_pool(name="gtp", bufs=1) as gtp:
            feats = []
            for p in range(2):
                Sg = gtp.tile([128, 2], F32, tag=f"Sg{p}")
                Ag = gtp.tile([128, 2], F32, tag=f"Ag{p}")
                Qg = gtp.tile([128, 2], F32, tag=f"Qg{p}")
                Mg = gtp.tile([128, 2], F32, tag=f"Mg{p}")
                for rb in range(2):
                    nc.vector.tensor_reduce(
                        Sg[:, rb:rb + 1], stats[(p, "se")][:, rb, :],
                        axis=X_AXIS, op=ALU.add)
                    nc.vector.tensor_reduce(
                        Ag[:, rb:rb + 1], stats[(p, "sa")][:, rb, :],
                        axis=X_AXIS, op=ALU.add)
                    nc.vector.tensor_reduce(
                        Qg[:, rb:rb + 1], stats[(p, "sq")][:, rb, :],
                        axis=X_AXIS, op=ALU.add)
                    nc.vector.tensor_reduce(
                        Mg[:, rb:rb + 1], stats[(p, "mx")][:, rb, :],
                        axis=X_AXIS, op=ALU.max)
                Lg = stats[(p, "sl")]
                rS = gtp.tile([128, 2], F32, tag=f"rS{p}")
                nc.vector.reciprocal(rS[:], Sg[:])
                AoS = gtp.tile([128, 2], F32, tag=f"AoS{p}")
                nc.vector.tensor_mul(AoS[:], Ag[:], rS[:])
                lnS = gtp.tile([128, 2], F32, tag=f"lnS{p}")
                nc.scalar.activation(lnS[:], Sg[:], AF.Ln)
                ent = gtp.tile([128, 2], F32, tag=f"ent{p}")
                nc.vector.tensor_sub(ent[:], lnS[:], AoS[:])
                mean = gtp.tile([128, 2], F32, tag=f"mean{p}")
                nc.vector.tensor_scalar_mul(mean[:], Lg[:], 1.0 / V)
                m2 = gtp.tile([128, 2], F32, tag=f"m2{p}")
                nc.vector.tensor_mul(m2[:], mean[:], mean[:])
                var = gtp.tile([128, 2], F32, tag=f"var{p}")
                nc.vector.tensor_scalar_mul(var[:], Qg[:], 1.0 / V)
                nc.vector.tensor_sub(var[:], var[:], m2[:])
                feats += [ent, Mg, var]

            acc = gtp.tile([128, 2], F32, tag="gacc")
            nc.vector.tensor_scalar_mul(acc[:], feats[0][:], float(gate_w[0]))
            for i in range(1, 6):
                nc.vector.scalar_tensor_tensor(
                    out=acc[:], in0=feats[i][:], scalar=float(gate_w[i]),
                    in1=acc[:], op0=ALU.mult, op1=ALU.add,
                )
            nc.scalar.activation(
                gate[:], acc[:], AF.Sigmoid, bias=float(gate_b), scale=1.0
            )
            if i8:
                # per-row |mix| bound: sqrt(max over paths/blocks of l^2);
                # mix is a convex combination so |mix| <= max(|lin|,|loc|)
                mq0 = gtp.tile([128, 2], F32, tag="mq0")
                mq1 = gtp.tile([128, 2], F32, tag="mq1")
                for rb in range(2):
                    nc.vector.tensor_reduce(
                        mq0[:, rb:rb + 1], stats[(0, "mq")][:, rb, :],
                        axis=X_AXIS, op=ALU.max)
                    nc.vector.tensor_reduce(
                        mq1[:, rb:rb + 1], stats[(1, "mq")][:, rb, :],
                        axis=X_AXIS, op=ALU.max)
                nc.vector.tensor_max(mq0[:], mq0[:], mq1[:])
                nc.scalar.activation(bound[:], mq0[:], AF.Sqrt)
                nc.sync.dma_start(out=sc_d[:], in_=bound[:])
                nc.vector.reciprocal(qscale[:], bound[:])
                nc.vector.tensor_scalar_mul(qscale[:], qscale[:], 126.0)

        # ---------- gated mix -> f16 output ----------
        with tc.tile_pool(name="mx", bufs=3) as mxp:
            for rb in range(2):
                for vb in range(NVB):
                    w = VBW if vb < NVB - 1 else VLAST
                    lin_s = mxp.tile([128, VBW], BF16, tag="lin")
                    loc_s = mxp.tile([128, VBW], BF16, tag="loc")
                    nc.sync.dma_start(
                        out=lin_s[:, :w], in_=ldram[0][rb, :, vb * VBW:vb * VBW + w])
                    nc.sync.dma_start(
                        out=loc_s[:, :w], in_=ldram[1][rb, :, vb * VBW:vb * VBW + w])
                    d = mxp.tile([128, VBW], BF16, tag="d")
                    nc.vector.tensor_sub(d[:, :w], lin_s[:, :w], loc_s[:, :w])
                    if i8:
                        t = mxp.tile([128, VBW], F32, tag="t")
                        nc.vector.scalar_tensor_tensor(
                            out=t[:, :w], in0=d[:, :w],
                            scalar=gate[:, rb:rb + 1],
                            in1=loc_s[:, :w], op0=ALU.mult, op1=ALU.add,
                        )
                        o = mxp.tile([128, VBW], out_dt, tag="o")
                        nc.vector.tensor_scalar_mul(
                            o[:, :w], t[:, :w], qscale[:, rb:rb + 1]
                        )
                    else:
                        o = mxp.tile([128, VBW], out_dt, tag="o")
                        nc.vector.scalar_tensor_tensor(
                            out=o[:, :w], in0=d[:, :w],
                            scalar=gate[:, rb:rb + 1],
                            in1=loc_s[:, :w], op0=ALU.mult, op1=ALU.add,
                        )
                    if kvar == "noout":
                        if rb == 0 and vb == 0:
                            nc.sync.dma_start(out=out_d[0, :, 0:16],
                                              in_=o[:, :16])
                    else:
                        nc.sync.dma_start(
                            out=out_d[rb, :, vb * VBW:vb * VBW + w],
                            in_=o[:, :w]
                        )

    nc.compile()
    return nc


def _pack_weights(in_proj, lin_W1, lin_b1, lin_W2, lin_b2,
                  loc_W1, loc_b1, loc_W2, loc_b2):
    def w2pad(w2, b2):
        w = np.zeros((H, VP), np.float32)
        w[:, :V] = np.asarray(w2, np.float32)
        w[:, V] = np.asarray(w2, np.float64).sum(axis=1)
        bp = np.zeros((VP,), np.float32)
        bp[:V] = np.asarray(b2, np.float32)
        bp[V] = np.asarray(b2, np.float64).sum()
        return _bf(w.reshape(HBLK, 128, VP)), bp

    w2l, b2l = w2pad(lin_W2, lin_b2)
    w2o, b2o = w2pad(loc_W2, loc_b2)
    use_b2 = bool(np.any(np.asarray(lin_b2)) or np.any(np.asarray(loc_b2)))
    return dict(
        inp=_bf(np.asarray(in_proj, np.float32).reshape(2, 128, M)),
        w1l=_bf(np.asarray(lin_W1, np.float32).reshape(4, 128, H)),
        w1o=_bf(np.asarray(loc_W1, np.float32).reshape(16, 128, H)),
        b1l=np.ascontiguousarray(
            np.asarray(lin_b1, np.float32).reshape(HBLK, 128, 1)),
        b1o=np.ascontiguousarray(
            np.asarray(loc_b1, np.float32).reshape(HBLK, 128, 1)),
        w2l=w2l, w2o=w2o,
        b2=_bf(np.stack([b2l, b2o]).reshape(2, 1, VP)),
        use_b2=use_b2,
    )


def prepare_inputs(chars, emb):
    """Host side: embedding gather + transpose/pad to xt, one-hot row masks."""
    x = np.asarray(emb, np.float32)[np.asarray(chars).astype(np.int64)]  # [B,T,E]
    xt = np.zeros((2, 128, CL), np.float32)
    for b in range(B):
        xTb = x[b].T  # [E, T]
        for e in range(2):
            xt[e, :, b * LPAD + W - 1:(b + 1) * LPAD] = \
                xTb[e * 128:(e + 1) * 128, :]
    xt_bf = np.ascontiguousarray(xt.astype(ml_dtypes.bfloat16))
    in_maps = []
    for c in range(N_CORES):
        msk = np.zeros((128, N_CORES), np.float32)
        msk[:, c] = 1.0
        in_maps.append(dict(xt=xt_bf, msk=msk))
    return in_maps


def assemble_output(results):
    parts = []
    for c in range(N_CORES):
        o = np.asarray(results[c]["out"])
        if o.dtype == np.int8:
            bound = np.asarray(results[c]["sc"], np.float32)  # [128, 2]
            scale = (bound.T.reshape(RPC, 1)) / 126.0         # row-major rb,p
            parts.append(o.astype(np.float32).reshape(RPC, V) * scale)
        else:
            parts.append(np.asarray(o, np.float32).reshape(RPC, V))
    rows = np.concatenate(parts, axis=0)
    return np.ascontiguousarray(rows.reshape(B, T, V))


_CACHE = {}


def _get_program(decays, gate_W, gate_b, in_proj, lin_W1, lin_b1, lin_W2,
                 lin_b2, loc_W1, loc_b1, loc_W2, loc_b2):
    import os
    key = (os.environ.get("KVAR", "full"),) + tuple(
        hash(np.ascontiguousarray(np.asarray(a)).tobytes())
        for a in (decays, gate_W, gate_b, in_proj, lin_W1, lin_b1, lin_W2,
                  lin_b2, loc_W1, loc_b1, loc_W2, loc_b2)
    )
    if key not in _CACHE:
        weights = _pack_weights(in_proj, lin_W1, lin_b1, lin_W2, lin_b2,
                                loc_W1, loc_b1, loc_W2, loc_b2)
        _CACHE[key] = build_program(
            np.asarray(decays, np.float32),
            np.asarray(gate_W, np.float64).reshape(-1),
            float(np.asarray(gate_b).reshape(-1)[0]),
            weights,
        )
    return _CACHE[key]


def get_nc_and_inmaps(inputs):
    nc = _get_program(
        inputs["decays"], inputs["gate_W"], inputs["gate_b"],
        inputs["in_proj"], inputs["lin_W1"], inputs["lin_b1"],
        inputs["lin_W2"], inputs["lin_b2"], inputs["loc_W1"],
        inputs["loc_b1"], inputs["loc_W2"], inputs["loc_b2"],
    )
    return nc, prepare_inputs(inputs["chars"], inputs["emb"])


def kernel(chars, emb, in_proj, decays, lin_W1, lin_b1, lin_W2, lin_b2,
           loc_W1, loc_b1, loc_W2, loc_b2, gate_W, gate_b):
    nc, in_maps = get_nc_and_inmaps(dict(
        chars=chars, emb=emb, in_proj=in_proj, decays=decays,
        lin_W1=lin_W1, lin_b1=lin_b1, lin_W2=lin_W2, lin_b2=lin_b2,
        loc_W1=loc_W1, loc_b1=loc_b1, loc_W2=loc_W2, loc_b2=loc_b2,
        gate_W=gate_W, gate_b=gate_b,
    ))
    res = run_bass_kernel_spmd(nc, in_maps, CORE_IDS)
    return assemble_output(res.results)
